# revision 46
# baseline (speedup 1.0000x reference)
"""Multi-head attention (B=2, S=2048, D=1024, H=16) on 8 NeuronCores.

Sharding: Megatron tensor parallelism. Core r owns heads 2r, 2r+1
(a 128-wide slice of D). Wq/Wk/Wv column-parallel. For the output
projection, attnT (each core's 128 D-rows, bf16) is exchanged in five
token-chunks with AllToAll so every core ends up with all 1024 D-rows
for a token slice; each core then multiplies by the full Wo and writes
its slice. Host reassembles the slices and adds bo.

All matmul operands are bf16 (1 cyc/row on the PE vs 4 for fp32);
PSUM accumulation stays fp32. Activations are converted to bf16 on
the host, halving HBM reads.

Scheduling notes:
 - attention is software-pipelined (scores for group n+1 are emitted
   before PV of group n) so exp never waits on the tensor queue;
 - batch 1's projections are interleaved piecewise into batch 0's
   attention stream, and the Wo chunks into batch 1's, keeping the PE
   continuously busy so it ramps to (and stays at) full clock;
 - weights load on the Activation DMA queue so the SP queue leads
   with the x tiles the first matmuls need.

Layouts on device (per core):
  xqT/xkT/xvT : [1024, 4096]  host-transposed bf16 activations
  qT/kT       : [128, 2048]   per batch, dk-major (rows = 2 heads)
  v           : [128, 130]    16 tok-tiles/batch; cols = [v_h0 | 1 | v_h1 | 1]
                              (ones column makes PV emit softmax sums)
  scores      : psum [128 sk, 1024] = 2 sk-tiles -> one Exp -> pt bf16
  PV          : psum [65, 512] accumulated over 16 sk tiles; row 64 = sums
  attnT       : [128, 2048]   normalized, per batch; AllToAll'd in chunks
"""

import sys

sys.path.insert(0, "/opt/trn_rl_repo")

import numpy as np

B, S, D, H, DK = 2, 2048, 1024, 16, 64
NCORES = 8
TOK = B * S            # 4096
DKC = D // NCORES      # 128 = 2 heads per core
TOKC = TOK // NCORES   # 512 output rows per core
KT = D // 128          # 8 contraction tiles
SKT = S // 128         # 16 key tiles per batch
SQB = S // 512         # 4 query blocks per batch
SKG = 2                # sk tiles per exp group
NG = SKT // SKG        # 8 exp groups per (head, query block)

# chunk c: (out_ext row offset, tokens per core). Chunks 0-2 cover 1024
# global tokens each, 3-4 cover 512 (the last exchange is small and the
# one before it overlaps the final attention block).
CHUNKS = [(0, 128), (128, 128), (256, 128), (384, 64), (448, 64)]
CHUNK_TOK0 = [0, 1024, 2048, 3072, 3584]

_cache = {}


def _build():
    from contextlib import ExitStack

    from concourse import bacc
    import concourse.mybir as mybir
    import concourse.tile as tile

    f32 = mybir.dt.float32
    bf16 = mybir.dt.bfloat16
    Act = mybir.ActivationFunctionType

    nc = bacc.Bacc(
        "TRN2", target_bir_lowering=False, debug=False,
        enable_asserts=False, num_devices=NCORES,
    )

    xqT = nc.dram_tensor("xqT", [D, TOK], bf16, kind="ExternalInput").ap()
    xkT = nc.dram_tensor("xkT", [D, TOK], bf16, kind="ExternalInput").ap()
    xvT = nc.dram_tensor("xvT", [D, TOK], bf16, kind="ExternalInput").ap()
    wq = nc.dram_tensor("wq", [D, DKC], bf16, kind="ExternalInput").ap()
    wk = nc.dram_tensor("wk", [D, DKC], bf16, kind="ExternalInput").ap()
    wv = nc.dram_tensor("wv", [D, DKC], bf16, kind="ExternalInput").ap()
    wo = nc.dram_tensor("wo", [D, D], bf16, kind="ExternalInput").ap()
    bq = nc.dram_tensor("bq", [DKC, 1], f32, kind="ExternalInput").ap()
    bk = nc.dram_tensor("bk", [DKC, 1], f32, kind="ExternalInput").ap()
    bv = nc.dram_tensor("bv", [1, DKC], bf16, kind="ExternalInput").ap()
    out_ext = nc.dram_tensor("out", [TOKC, D], f32, kind="ExternalOutput").ap()

    with tile.TileContext(nc) as tc, ExitStack() as ctx, \
            nc.allow_low_precision("bf16 matmul operands, fp32 psum accumulate"):
        wpool = ctx.enter_context(tc.tile_pool(name="w", bufs=1))
        xpool = ctx.enter_context(tc.tile_pool(name="x", bufs=3))
        qkpool = ctx.enter_context(tc.tile_pool(name="qk", bufs=2))
        vpool = ctx.enter_context(tc.tile_pool(name="v", bufs=2))
        ptpool = ctx.enter_context(tc.tile_pool(name="pt", bufs=3))
        atpool = ctx.enter_context(tc.tile_pool(name="at", bufs=2))
        smpool = ctx.enter_context(tc.tile_pool(name="sm", bufs=2))
        agpool = ctx.enter_context(tc.tile_pool(name="ag", bufs=1))
        opool = ctx.enter_context(tc.tile_pool(name="o", bufs=2))
        ps_g = ctx.enter_context(tc.tile_pool(name="psg", bufs=2, space="PSUM"))
        ps_mm = ctx.enter_context(tc.tile_pool(name="psmm", bufs=2, space="PSUM"))
        ps_acc = ctx.enter_context(tc.tile_pool(name="psacc", bufs=2, space="PSUM"))
        dram = ctx.enter_context(tc.tile_pool(name="dram", bufs=1, space="DRAM"))

        # ---- constants / weights into SBUF (Activation DMA queue) ----
        bq_t = wpool.tile([DKC, 1], f32, tag="bq")
        nc.scalar.dma_start(bq_t[:], bq[:])
        bk_t = wpool.tile([DKC, 1], f32, tag="bk")
        nc.scalar.dma_start(bk_t[:], bk[:])
        bv_t = wpool.tile([1, DKC], bf16, tag="bv")
        nc.scalar.dma_start(bv_t[:], bv[:])
        wq_t, wk_t, wv_t = [], [], []
        for name, src, lst in (("wq", wq, wq_t), ("wk", wk, wk_t), ("wv", wv, wv_t)):
            for k in range(KT):
                t = wpool.tile([128, DKC], bf16, tag=f"{name}{k}")
                nc.scalar.dma_start(t[:], src[k * 128:(k + 1) * 128, :])
                lst.append(t)
        wo_t = []
        for r in range(NCORES):
            t = wpool.tile([128, D], bf16, tag=f"wo{r}")
            nc.scalar.dma_start(t[:], wo[r * 128:(r + 1) * 128, :])
            wo_t.append(t)
        ones_t = wpool.tile([1, 128], bf16, tag="ones")
        nc.gpsimd.memset(ones_t[:], 1.0)
        ones_f = wpool.tile([1, 64], f32, tag="onesf")
        nc.gpsimd.memset(ones_f[:], 1.0)

        # ---------- emission helpers ----------

        def emit_x_load(xT, b, k, out_list):
            t = xpool.tile([128, S], bf16, tag=f"x{k}")
            nc.sync.dma_start(t[:], xT[k * 128:(k + 1) * 128, b * S:(b + 1) * S])
            out_list[k] = t

        def emit_qk_chain(chst, xts, w_list, bias_t, dst, blk, part):
            """Half of one 512-column projection chain (4 of 8 k-steps);
            part 0 allocates the psum tile, part 1 drains it."""
            if part == 0:
                chst[blk] = ps_mm.tile(
                    [128, 512], f32, tag="mm", name=f"chain{blk}")
            ps = chst[blk]
            for k in range(part * 4, part * 4 + 4):
                nc.tensor.matmul(
                    ps[:], lhsT=w_list[k][:],
                    rhs=xts[k][:, blk * 512:(blk + 1) * 512],
                    start=(k == 0), stop=(k == KT - 1),
                )
            if part == 1:
                nc.vector.tensor_scalar_add(
                    dst[:, blk * 512:(blk + 1) * 512], ps[:], bias_t[:, 0:1])

        def emit_v_tile(xvs, mi, v_tiles):
            ps = ps_mm.tile([128, DKC], f32, tag="mm")
            for k in range(KT):
                nc.tensor.matmul(
                    ps[:], lhsT=xvs[k][:, mi * 128:(mi + 1) * 128],
                    rhs=wv_t[k][:], start=(k == 0), stop=False,
                )
            nc.tensor.matmul(
                ps[:], lhsT=ones_t[0:1, :], rhs=bv_t[:],
                start=False, stop=True,
            )
            vt = vpool.tile([128, 130], bf16, tag=f"v{mi}")
            nc.vector.tensor_copy(vt[:, 0:64], ps[:, 0:64])
            nc.vector.tensor_copy(vt[:, 65:129], ps[:, 64:128])
            nc.vector.memset(vt[:, 64:65], 1.0)
            nc.vector.memset(vt[:, 129:130], 1.0)
            v_tiles[mi] = vt

        def proj_pieces(b, st):
            """Piecewise emission of batch b's x loads + q/k/v projections.
            Returns a list of (min_idx, thunk); thunks must run in order."""
            st["q"] = qkpool.tile([128, S], bf16, tag="qT", name=f"qT{b}")
            st["k"] = qkpool.tile([128, S], bf16, tag="kT", name=f"kT{b}")
            st["v"] = [None] * SKT
            xq_l, xk_l, xv_l = [None] * KT, [None] * KT, [None] * KT
            ps = []

            def loads(xT, lst):
                def mk(k):
                    return lambda: emit_x_load(xT, b, k, lst)
                return [mk(k) for k in range(KT)]

            chq, chk = {}, {}
            ps += loads(xqT, xq_l)
            ps += loads(xkT, xk_l)
            for blk in range(SQB):
                for part in range(2):
                    ps.append(lambda blk=blk, part=part: emit_qk_chain(
                        chq, xq_l, wq_t, bq_t, st["q"], blk, part))
            ps += loads(xvT, xv_l)
            for blk in range(SQB):
                for part in range(2):
                    ps.append(lambda blk=blk, part=part: emit_qk_chain(
                        chk, xk_l, wk_t, bk_t, st["k"], blk, part))
            for mi in range(SKT):
                ps.append(lambda mi=mi: emit_v_tile(xv_l, mi, st["v"]))
            # spread pieces roughly one per attention group
            return [(j, f) for j, f in enumerate(ps)]

        def emit_exchange(c, attnT):
            tokpc = CHUNKS[c][1]
            loc0 = CHUNK_TOK0[c] % S
            ain = dram.tile([NCORES * 128, tokpc], bf16, tag=f"a2ai{c}")
            aout = dram.tile([NCORES * 128, tokpc], bf16, tag=f"a2ao{c}")
            for j in range(NCORES):
                nc.sync.dma_start(
                    ain[j * 128:(j + 1) * 128, :],
                    attnT[:, loc0 + j * tokpc: loc0 + (j + 1) * tokpc],
                )
            nc.gpsimd.collective_compute(
                "AllToAll",
                mybir.AluOpType.bypass,
                replica_groups=[list(range(NCORES))],
                ins=[ain.opt()],
                outs=[aout.opt()],
            )
            return aout

        def wo_pieces(c, aouts, st, nparts=2):
            """agt loads, then each output half as `nparts` psum chain
            parts (8/nparts accumulation matmuls each)."""
            roff, tokpc = CHUNKS[c]
            rstep = NCORES // nparts

            def p_load():
                agts = []
                for r in range(NCORES):
                    t = agpool.tile([128, tokpc], bf16, tag=f"ag{c}_{r}")
                    nc.sync.dma_start(
                        t[:], aouts[c][r * 128:(r + 1) * 128, :])
                    agts.append(t)
                st[c] = agts

            def p_half(half, part):
                agts = st[c]
                if part == 0:
                    st[(c, half)] = ps_mm.tile(
                        [tokpc, 512], f32, tag="mm", name=f"wo{c}_{half}")
                ps = st[(c, half)]
                for r in range(part * rstep, (part + 1) * rstep):
                    nc.tensor.matmul(
                        ps[:], lhsT=agts[r][:],
                        rhs=wo_t[r][:, half * 512:(half + 1) * 512],
                        start=(r == 0), stop=(r == NCORES - 1),
                    )
                if part == nparts - 1:
                    ot = opool.tile([tokpc, 512], f32, tag="ot")
                    nc.vector.tensor_copy(ot[:], ps[:])
                    nc.sync.dma_start(
                        out_ext[roff:roff + tokpc,
                                half * 512:(half + 1) * 512],
                        ot[:],
                    )

            return [p_load] + [
                (lambda half=half, part=part: p_half(half, part))
                for half in range(2) for part in range(nparts)]

        def emit_attention(b, qT_b, kT_b, v_tiles, filler):
            """Software-pipelined attention for batch b. `filler` is a list
            of (min_idx, thunk); thunks are popped in order, at most one
            per group, once idx >= min_idx."""
            attnT = atpool.tile([128, S], bf16, tag="attnT")
            items = [(sq, h, g)
                     for sq in range(SQB) for h in range(2) for g in range(NG)]

            def emit_scores(it):
                sq, h, g = it
                hp = h * 64
                qs = slice(sq * 512, (sq + 1) * 512)
                sg = ps_g.tile([128, 512 * SKG], f32, tag="sg")
                for i in range(SKG):
                    sk = g * SKG + i
                    nc.tensor.matmul(
                        sg[:, i * 512:(i + 1) * 512],
                        lhsT=kT_b[hp:hp + 64, sk * 128:(sk + 1) * 128],
                        rhs=qT_b[hp:hp + 64, qs],
                        start=True, stop=True,
                    )
                return sg

            sgs = {items[0]: emit_scores(items[0])}
            xps = None
            for idx, it in enumerate(items):
                sq, h, g = it
                hp = h * 64
                if idx + 1 < len(items):
                    sgs[items[idx + 1]] = emit_scores(items[idx + 1])
                sg = sgs.pop(it)
                ptg = ptpool.tile([128, 512 * SKG], bf16, tag="pt")
                nc.scalar.activation(ptg[:], sg[:], Act.Exp, scale=0.125)
                # filler slots in here: it runs on the tensor engine while
                # ACT computes the exp that the PV pair below waits on
                while filler and idx >= filler[0][0]:
                    filler.pop(0)[1]()
                if g == 0:
                    xps = ps_acc.tile([65, 512], f32, tag="acc")
                for i in range(SKG):
                    sk = g * SKG + i
                    nc.tensor.matmul(
                        xps[:],
                        lhsT=v_tiles[sk][:, h * 65:h * 65 + 65],
                        rhs=ptg[:, i * 512:(i + 1) * 512],
                        start=(sk == 0), stop=(sk == SKT - 1),
                    )
                if g == NG - 1:
                    # bf16 broadcast matmul (1 cyc/row vs 4 for fp32); the
                    # bf16-rounded denominator adds <1e-3 to the error
                    rowsum = smpool.tile([1, 512], bf16, tag="rs")
                    nc.scalar.activation(rowsum[:], xps[64:65, :], Act.Identity)
                    rbp = ps_mm.tile([64, 512], f32, tag="mm")
                    nc.tensor.matmul(
                        rbp[:], lhsT=ones_t[0:1, 0:64], rhs=rowsum[:],
                        start=True, stop=True,
                    )
                    rb = smpool.tile([64, 512], f32, tag="rb")
                    nc.vector.reciprocal_approx_fast(rb[:], rbp[:])
                    nc.vector.tensor_mul(
                        attnT[hp:hp + 64, sq * 512:(sq + 1) * 512],
                        xps[0:64, :], rb[:],
                    )
                    if h == 1 and (b, sq) in EXCH_AFTER:
                        for c in EXCH_AFTER[(b, sq)]:
                            aouts[c] = emit_exchange(c, attnT)
            # drain leftover filler
            for _, f in filler:
                f()
            filler.clear()

        # exchange c is emitted right after the attention block that
        # completes its tokens: c0 after (b0,sq1), c1 after (b0,sq3),
        # c2 after (b1,sq1), c3 after (b1,sq2), c4 after (b1,sq3).
        EXCH_AFTER = {(0, 1): [0], (0, 3): [1],
                      (1, 1): [2], (1, 2): [3], (1, 3): [4]}
        aouts = {}

        # ---- batch 0 x loads + q/k projections (inline lead-in); its
        # v projection runs as early just-in-time filler inside its own
        # attention (PV of group g needs v tile 2g+1, and two v pieces
        # are popped per group), followed by batch 1's pieces ----
        st0 = {}
        p0 = proj_pieces(0, st0)
        nv = SKT  # trailing SKT pieces of proj_pieces are the v tiles
        for _, f in p0[:-nv]:
            f()
        st1 = {}
        filler0 = [(j // 2, f) for j, (_, f) in enumerate(p0[-nv:])]
        filler0 += [(8 + j, f) for j, (_, f) in enumerate(proj_pieces(1, st1))]
        emit_attention(0, st0["q"], st0["k"], st0["v"], filler0)

        # ---- batch 1 attention, interleaving the Wo chunks as fine
        # 2-matmul pieces: spreading them over more groups keeps the PE
        # continuously busy so it holds full clock through the window ----
        agst = {}
        filler = []
        for j, f in enumerate(wo_pieces(0, aouts, agst, nparts=4)):
            filler.append((j * 2, f))
        for j, f in enumerate(wo_pieces(1, aouts, agst, nparts=4)):
            filler.append((18 + j * 2, f))
        # chunk 2's exchange is emitted after (b1, sq1) = idx 31
        for j, f in enumerate(wo_pieces(2, aouts, agst, nparts=4)):
            filler.append((40 + j * 2, f))
        emit_attention(1, st1["q"], st1["k"], st1["v"], filler)

        # ---- last two chunks' output projections (tail; chunk 3's
        # collective completed during the final attention block) ----
        for f in wo_pieces(3, aouts, agst):
            f()
        for f in wo_pieces(4, aouts, agst):
            f()

    nc.compile()
    return nc


def _get_nc():
    if "nc" not in _cache:
        _cache["nc"] = _build()
    return _cache["nc"]


def kernel(query, key, value, Wq, bq, Wk, bk, Wv, bv, Wo, bo, trace=False):
    import ml_dtypes
    from concourse.bass_utils import run_bass_kernel_spmd

    bf = ml_dtypes.bfloat16
    nc = _get_nc()

    q = np.ascontiguousarray(
        np.asarray(query, np.float32).reshape(TOK, D).T.astype(bf))
    k = np.ascontiguousarray(
        np.asarray(key, np.float32).reshape(TOK, D).T.astype(bf))
    v = np.ascontiguousarray(
        np.asarray(value, np.float32).reshape(TOK, D).T.astype(bf))
    Wq = np.asarray(Wq, np.float32)
    Wk = np.asarray(Wk, np.float32)
    Wv = np.asarray(Wv, np.float32)
    Wo_b = np.ascontiguousarray(np.asarray(Wo, np.float32).astype(bf))

    in_maps = []
    for r in range(NCORES):
        sl = slice(r * DKC, (r + 1) * DKC)
        in_maps.append({
            "xqT": q, "xkT": k, "xvT": v,
            "wq": np.ascontiguousarray(Wq[:, sl].astype(bf)),
            "wk": np.ascontiguousarray(Wk[:, sl].astype(bf)),
            "wv": np.ascontiguousarray(Wv[:, sl].astype(bf)),
            "wo": Wo_b,
            "bq": np.ascontiguousarray(np.asarray(bq, np.float32)[sl, None]),
            "bk": np.ascontiguousarray(np.asarray(bk, np.float32)[sl, None]),
            "bv": np.ascontiguousarray(
                np.asarray(bv, np.float32)[None, sl].astype(bf)),
        })

    res = run_bass_kernel_spmd(nc, in_maps, list(range(NCORES)), trace=trace)
    _cache["last_results"] = res

    out = np.empty((TOK, D), np.float32)
    for r in range(NCORES):
        o = np.asarray(res.results[r]["out"])
        for c, (roff, tokpc) in enumerate(CHUNKS):
            g0 = CHUNK_TOK0[c]
            out[g0 + r * tokpc: g0 + (r + 1) * tokpc] = \
                o[roff:roff + tokpc]
    out = out + np.asarray(bo, np.float32)[None, :]
    return out.reshape(B, S, D)


# revision 47
# speedup vs baseline: 1.1152x; 1.1152x over previous
"""Multi-head attention (B=2, S=2048, D=1024, H=16) on 8 NeuronCores.

Sharding: Megatron tensor parallelism. Core r owns heads 2r, 2r+1
(a 128-wide slice of D). Wq/Wk/Wv column-parallel. For the output
projection, attnT (each core's 128 D-rows, bf16) is exchanged in five
token-chunks with AllToAll so every core ends up with all 1024 D-rows
for a token slice; each core then multiplies by the full Wo and writes
its slice. Host reassembles the slices and adds bo.

All matmul operands are bf16 (1 cyc/row on the PE vs 4 for fp32);
PSUM accumulation stays fp32. Activations are converted to bf16 on
the host, halving HBM reads.

Scheduling notes:
 - attention is software-pipelined (scores for group n+1 are emitted
   before PV of group n) so exp never waits on the tensor queue;
 - batch 1's projections are interleaved piecewise into batch 0's
   attention stream, and the Wo chunks into batch 1's, keeping the PE
   continuously busy so it ramps to (and stays at) full clock;
 - weights load on the Activation DMA queue so the SP queue leads
   with the x tiles the first matmuls need.

Layouts on device (per core):
  xqT/xkT/xvT : [1024, 4096]  host-transposed bf16 activations
  qT/kT       : [128, 2048]   per batch, dk-major (rows = 2 heads)
  v           : [128, 130]    16 tok-tiles/batch; cols = [v_h0 | 1 | v_h1 | 1]
                              (ones column makes PV emit softmax sums)
  scores      : psum [128 sk, 1024] = 2 sk-tiles -> one Exp -> pt bf16
  PV          : psum [65, 512] accumulated over 16 sk tiles; row 64 = sums
  attnT       : [128, 2048]   normalized, per batch; AllToAll'd in chunks
"""

import sys

sys.path.insert(0, "/opt/trn_rl_repo")

import numpy as np

B, S, D, H, DK = 2, 2048, 1024, 16, 64
NCORES = 8
TOK = B * S            # 4096
DKC = D // NCORES      # 128 = 2 heads per core
TOKC = TOK // NCORES   # 512 output rows per core
KT = D // 128          # 8 contraction tiles
SKT = S // 128         # 16 key tiles per batch
SQB = S // 512         # 4 query blocks per batch
SKG = 2                # sk tiles per exp group
NG = SKT // SKG        # 8 exp groups per (head, query block)

# chunk c: (out_ext row offset, tokens per core). Chunks 0-2 cover 1024
# global tokens each, 3-4 cover 512 (the last exchange is small and the
# one before it overlaps the final attention block).
CHUNKS = [(0, 128), (128, 128), (256, 128), (384, 64), (448, 64)]
CHUNK_TOK0 = [0, 1024, 2048, 3072, 3584]

_cache = {}


def _build():
    from contextlib import ExitStack

    from concourse import bacc
    import concourse.mybir as mybir
    import concourse.tile as tile

    f32 = mybir.dt.float32
    bf16 = mybir.dt.bfloat16
    Act = mybir.ActivationFunctionType

    nc = bacc.Bacc(
        "TRN2", target_bir_lowering=False, debug=False,
        enable_asserts=False, num_devices=NCORES,
    )

    xqT = nc.dram_tensor("xqT", [D, TOK], bf16, kind="ExternalInput").ap()
    xkT = nc.dram_tensor("xkT", [D, TOK], bf16, kind="ExternalInput").ap()
    xvT = nc.dram_tensor("xvT", [D, TOK], bf16, kind="ExternalInput").ap()
    wq = nc.dram_tensor("wq", [D, DKC], bf16, kind="ExternalInput").ap()
    wk = nc.dram_tensor("wk", [D, DKC], bf16, kind="ExternalInput").ap()
    wv = nc.dram_tensor("wv", [D, DKC], bf16, kind="ExternalInput").ap()
    wo = nc.dram_tensor("wo", [D, D], bf16, kind="ExternalInput").ap()
    bq = nc.dram_tensor("bq", [DKC, 1], f32, kind="ExternalInput").ap()
    bk = nc.dram_tensor("bk", [DKC, 1], f32, kind="ExternalInput").ap()
    bv = nc.dram_tensor("bv", [1, DKC], bf16, kind="ExternalInput").ap()
    out_ext = nc.dram_tensor("out", [TOKC, D], f32, kind="ExternalOutput").ap()

    with tile.TileContext(nc) as tc, ExitStack() as ctx, \
            nc.allow_low_precision("bf16 matmul operands, fp32 psum accumulate"):
        wpool = ctx.enter_context(tc.tile_pool(name="w", bufs=1))
        xpool = ctx.enter_context(tc.tile_pool(name="x", bufs=2))
        qkpool = ctx.enter_context(tc.tile_pool(name="qk", bufs=2))
        vpool = ctx.enter_context(tc.tile_pool(name="v", bufs=2))
        ptpool = ctx.enter_context(tc.tile_pool(name="pt", bufs=3))
        atpool = ctx.enter_context(tc.tile_pool(name="at", bufs=2))
        smpool = ctx.enter_context(tc.tile_pool(name="sm", bufs=2))
        agpool = ctx.enter_context(tc.tile_pool(name="ag", bufs=1))
        opool = ctx.enter_context(tc.tile_pool(name="o", bufs=2))
        ps_g = ctx.enter_context(tc.tile_pool(name="psg", bufs=2, space="PSUM"))
        ps_mm = ctx.enter_context(tc.tile_pool(name="psmm", bufs=2, space="PSUM"))
        ps_acc = ctx.enter_context(tc.tile_pool(name="psacc", bufs=2, space="PSUM"))
        dram = ctx.enter_context(tc.tile_pool(name="dram", bufs=1, space="DRAM"))

        # ---- constants / weights into SBUF (Activation DMA queue) ----
        bq_t = wpool.tile([DKC, 1], f32, tag="bq")
        nc.scalar.dma_start(bq_t[:], bq[:])
        bk_t = wpool.tile([DKC, 1], f32, tag="bk")
        nc.scalar.dma_start(bk_t[:], bk[:])
        bv_t = wpool.tile([1, DKC], bf16, tag="bv")
        nc.scalar.dma_start(bv_t[:], bv[:])
        wq_t, wk_t, wv_t = [], [], []
        for name, src, lst in (("wq", wq, wq_t), ("wk", wk, wk_t), ("wv", wv, wv_t)):
            for k in range(KT):
                t = wpool.tile([128, DKC], bf16, tag=f"{name}{k}")
                nc.scalar.dma_start(t[:], src[k * 128:(k + 1) * 128, :])
                lst.append(t)
        wo_t = []
        for r in range(NCORES):
            t = wpool.tile([128, D], bf16, tag=f"wo{r}")
            nc.scalar.dma_start(t[:], wo[r * 128:(r + 1) * 128, :])
            wo_t.append(t)
        ones_t = wpool.tile([1, 128], bf16, tag="ones")
        nc.gpsimd.memset(ones_t[:], 1.0)
        ones_f = wpool.tile([1, 64], f32, tag="onesf")
        nc.gpsimd.memset(ones_f[:], 1.0)

        # ---------- emission helpers ----------

        def emit_x_load(xT, b, k, out_list):
            t = xpool.tile([128, S], bf16, tag=f"x{k}")
            nc.sync.dma_start(t[:], xT[k * 128:(k + 1) * 128, b * S:(b + 1) * S])
            out_list[k] = t

        def emit_qk_chain(chst, xts, w_list, bias_t, dst, blk, part):
            """Half of one 512-column projection chain (4 of 8 k-steps);
            part 0 allocates the psum tile, part 1 drains it."""
            if part == 0:
                chst[blk] = ps_mm.tile(
                    [128, 512], f32, tag="mm", name=f"chain{blk}")
            ps = chst[blk]
            for k in range(part * 4, part * 4 + 4):
                nc.tensor.matmul(
                    ps[:], lhsT=w_list[k][:],
                    rhs=xts[k][:, blk * 512:(blk + 1) * 512],
                    start=(k == 0), stop=(k == KT - 1),
                )
            if part == 1:
                nc.vector.tensor_scalar_add(
                    dst[:, blk * 512:(blk + 1) * 512], ps[:], bias_t[:, 0:1])

        def emit_v_tile(xvs, mi, v_tiles):
            ps = ps_mm.tile([128, DKC], f32, tag="mm")
            for k in range(KT):
                nc.tensor.matmul(
                    ps[:], lhsT=xvs[k][:, mi * 128:(mi + 1) * 128],
                    rhs=wv_t[k][:], start=(k == 0), stop=False,
                )
            nc.tensor.matmul(
                ps[:], lhsT=ones_t[0:1, :], rhs=bv_t[:],
                start=False, stop=True,
            )
            vt = vpool.tile([128, 130], bf16, tag=f"v{mi}")
            nc.vector.tensor_copy(vt[:, 0:64], ps[:, 0:64])
            nc.vector.tensor_copy(vt[:, 65:129], ps[:, 64:128])
            nc.vector.memset(vt[:, 64:65], 1.0)
            nc.vector.memset(vt[:, 129:130], 1.0)
            v_tiles[mi] = vt

        def proj_pieces(b, st):
            """Piecewise emission of batch b's x loads + q/k/v projections.
            Returns a list of (min_idx, thunk); thunks must run in order."""
            st["q"] = qkpool.tile([128, S], bf16, tag="qT", name=f"qT{b}")
            st["k"] = qkpool.tile([128, S], bf16, tag="kT", name=f"kT{b}")
            st["v"] = [None] * SKT
            xq_l, xk_l, xv_l = [None] * KT, [None] * KT, [None] * KT
            ps = []

            def loads(xT, lst):
                def mk(k):
                    return lambda: emit_x_load(xT, b, k, lst)
                return [mk(k) for k in range(KT)]

            chq, chk = {}, {}
            ps += loads(xqT, xq_l)
            ps += loads(xkT, xk_l)
            for blk in range(SQB):
                for part in range(2):
                    ps.append(lambda blk=blk, part=part: emit_qk_chain(
                        chq, xq_l, wq_t, bq_t, st["q"], blk, part))
            ps += loads(xvT, xv_l)
            for blk in range(SQB):
                for part in range(2):
                    ps.append(lambda blk=blk, part=part: emit_qk_chain(
                        chk, xk_l, wk_t, bk_t, st["k"], blk, part))
            for mi in range(SKT):
                ps.append(lambda mi=mi: emit_v_tile(xv_l, mi, st["v"]))
            # spread pieces roughly one per attention group
            return [(j, f) for j, f in enumerate(ps)]

        def emit_exchange(c, attnT):
            tokpc = CHUNKS[c][1]
            loc0 = CHUNK_TOK0[c] % S
            ain = dram.tile([NCORES * 128, tokpc], bf16, tag=f"a2ai{c}")
            aout = dram.tile([NCORES * 128, tokpc], bf16, tag=f"a2ao{c}")
            for j in range(NCORES):
                nc.sync.dma_start(
                    ain[j * 128:(j + 1) * 128, :],
                    attnT[:, loc0 + j * tokpc: loc0 + (j + 1) * tokpc],
                )
            nc.gpsimd.collective_compute(
                "AllToAll",
                mybir.AluOpType.bypass,
                replica_groups=[list(range(NCORES))],
                ins=[ain.opt()],
                outs=[aout.opt()],
            )
            return aout

        def wo_pieces(c, aouts, st, nparts=2):
            """agt loads, then each output half as `nparts` psum chain
            parts (8/nparts accumulation matmuls each)."""
            roff, tokpc = CHUNKS[c]
            rstep = NCORES // nparts

            def p_load():
                agts = []
                for r in range(NCORES):
                    t = agpool.tile([128, tokpc], bf16, tag=f"ag{c}_{r}")
                    nc.sync.dma_start(
                        t[:], aouts[c][r * 128:(r + 1) * 128, :])
                    agts.append(t)
                st[c] = agts

            def p_half(half, part):
                agts = st[c]
                if part == 0:
                    st[(c, half)] = ps_mm.tile(
                        [tokpc, 512], f32, tag="mm", name=f"wo{c}_{half}")
                ps = st[(c, half)]
                for r in range(part * rstep, (part + 1) * rstep):
                    nc.tensor.matmul(
                        ps[:], lhsT=agts[r][:],
                        rhs=wo_t[r][:, half * 512:(half + 1) * 512],
                        start=(r == 0), stop=(r == NCORES - 1),
                    )
                if part == nparts - 1:
                    ot = opool.tile([tokpc, 512], f32, tag="ot")
                    nc.vector.tensor_copy(ot[:], ps[:])
                    nc.sync.dma_start(
                        out_ext[roff:roff + tokpc,
                                half * 512:(half + 1) * 512],
                        ot[:],
                    )

            return [p_load] + [
                (lambda half=half, part=part: p_half(half, part))
                for half in range(2) for part in range(nparts)]

        def emit_attention(b, qT_b, kT_b, v_tiles, filler):
            """Software-pipelined attention for batch b. `filler` is a list
            of (min_idx, thunk); thunks are popped in order, at most one
            per group, once idx >= min_idx."""
            attnT = atpool.tile([128, S], bf16, tag="attnT")
            items = [(sq, h, g)
                     for sq in range(SQB) for h in range(2) for g in range(NG)]

            def emit_scores(it):
                sq, h, g = it
                hp = h * 64
                qs = slice(sq * 512, (sq + 1) * 512)
                sg = ps_g.tile([128, 512 * SKG], f32, tag="sg")
                for i in range(SKG):
                    sk = g * SKG + i
                    nc.tensor.matmul(
                        sg[:, i * 512:(i + 1) * 512],
                        lhsT=kT_b[hp:hp + 64, sk * 128:(sk + 1) * 128],
                        rhs=qT_b[hp:hp + 64, qs],
                        start=True, stop=True,
                    )
                return sg

            sgs = {items[0]: emit_scores(items[0])}
            xps = None
            for idx, it in enumerate(items):
                sq, h, g = it
                hp = h * 64
                if idx + 1 < len(items):
                    sgs[items[idx + 1]] = emit_scores(items[idx + 1])
                sg = sgs.pop(it)
                ptg = ptpool.tile([128, 512 * SKG], bf16, tag="pt")
                nc.scalar.activation(ptg[:], sg[:], Act.Exp, scale=0.125)
                # filler slots in here: it runs on the tensor engine while
                # ACT computes the exp that the PV pair below waits on
                while filler and idx >= filler[0][0]:
                    filler.pop(0)[1]()
                if g == 0:
                    xps = ps_acc.tile([65, 512], f32, tag="acc")
                for i in range(SKG):
                    sk = g * SKG + i
                    nc.tensor.matmul(
                        xps[:],
                        lhsT=v_tiles[sk][:, h * 65:h * 65 + 65],
                        rhs=ptg[:, i * 512:(i + 1) * 512],
                        start=(sk == 0), stop=(sk == SKT - 1),
                    )
                if g == NG - 1:
                    # bf16 broadcast matmul (1 cyc/row vs 4 for fp32); the
                    # bf16-rounded denominator adds <1e-3 to the error
                    rowsum = smpool.tile([1, 512], bf16, tag="rs")
                    nc.scalar.activation(rowsum[:], xps[64:65, :], Act.Identity)
                    rbp = ps_mm.tile([64, 512], f32, tag="mm")
                    nc.tensor.matmul(
                        rbp[:], lhsT=ones_t[0:1, 0:64], rhs=rowsum[:],
                        start=True, stop=True,
                    )
                    rb = smpool.tile([64, 512], f32, tag="rb")
                    nc.vector.reciprocal_approx_fast(rb[:], rbp[:])
                    nc.vector.tensor_mul(
                        attnT[hp:hp + 64, sq * 512:(sq + 1) * 512],
                        xps[0:64, :], rb[:],
                    )
                    if h == 1 and (b, sq) in EXCH_AFTER:
                        for c in EXCH_AFTER[(b, sq)]:
                            aouts[c] = emit_exchange(c, attnT)
            # drain leftover filler
            for _, f in filler:
                f()
            filler.clear()

        # exchange c is emitted right after the attention block that
        # completes its tokens: c0 after (b0,sq1), c1 after (b0,sq3),
        # c2 after (b1,sq1), c3 after (b1,sq2), c4 after (b1,sq3).
        EXCH_AFTER = {(0, 1): [0], (0, 3): [1],
                      (1, 1): [2], (1, 2): [3], (1, 3): [4]}
        aouts = {}

        # ---- batch 0 x loads + q/k projections (inline lead-in); its
        # v projection runs as early just-in-time filler inside its own
        # attention (PV of group g needs v tile 2g+1, and two v pieces
        # are popped per group), followed by batch 1's pieces ----
        st0 = {}
        p0 = proj_pieces(0, st0)
        nv = SKT  # trailing SKT pieces of proj_pieces are the v tiles
        for _, f in p0[:-nv]:
            f()
        st1 = {}
        filler0 = [(j // 2, f) for j, (_, f) in enumerate(p0[-nv:])]
        filler0 += [(8 + j, f) for j, (_, f) in enumerate(proj_pieces(1, st1))]
        emit_attention(0, st0["q"], st0["k"], st0["v"], filler0)

        # ---- batch 1 attention, interleaving the Wo chunks as fine
        # 2-matmul pieces: spreading them over more groups keeps the PE
        # continuously busy so it holds full clock through the window ----
        agst = {}
        filler = []
        for j, f in enumerate(wo_pieces(0, aouts, agst, nparts=4)):
            filler.append((j * 2, f))
        for j, f in enumerate(wo_pieces(1, aouts, agst, nparts=4)):
            filler.append((18 + j * 2, f))
        # chunk 2's exchange is emitted after (b1, sq1) = idx 31
        for j, f in enumerate(wo_pieces(2, aouts, agst, nparts=4)):
            filler.append((40 + j * 2, f))
        emit_attention(1, st1["q"], st1["k"], st1["v"], filler)

        # ---- last two chunks' output projections (tail; chunk 3's
        # collective completed during the final attention block) ----
        for f in wo_pieces(3, aouts, agst):
            f()
        for f in wo_pieces(4, aouts, agst):
            f()

    nc.compile()
    return nc


def _get_nc():
    if "nc" not in _cache:
        _cache["nc"] = _build()
    return _cache["nc"]


def kernel(query, key, value, Wq, bq, Wk, bk, Wv, bv, Wo, bo, trace=False):
    import ml_dtypes
    from concourse.bass_utils import run_bass_kernel_spmd

    bf = ml_dtypes.bfloat16
    nc = _get_nc()

    q = np.ascontiguousarray(
        np.asarray(query, np.float32).reshape(TOK, D).T.astype(bf))
    k = np.ascontiguousarray(
        np.asarray(key, np.float32).reshape(TOK, D).T.astype(bf))
    v = np.ascontiguousarray(
        np.asarray(value, np.float32).reshape(TOK, D).T.astype(bf))
    Wq = np.asarray(Wq, np.float32)
    Wk = np.asarray(Wk, np.float32)
    Wv = np.asarray(Wv, np.float32)
    Wo_b = np.ascontiguousarray(np.asarray(Wo, np.float32).astype(bf))

    in_maps = []
    for r in range(NCORES):
        sl = slice(r * DKC, (r + 1) * DKC)
        in_maps.append({
            "xqT": q, "xkT": k, "xvT": v,
            "wq": np.ascontiguousarray(Wq[:, sl].astype(bf)),
            "wk": np.ascontiguousarray(Wk[:, sl].astype(bf)),
            "wv": np.ascontiguousarray(Wv[:, sl].astype(bf)),
            "wo": Wo_b,
            "bq": np.ascontiguousarray(np.asarray(bq, np.float32)[sl, None]),
            "bk": np.ascontiguousarray(np.asarray(bk, np.float32)[sl, None]),
            "bv": np.ascontiguousarray(
                np.asarray(bv, np.float32)[None, sl].astype(bf)),
        })

    res = run_bass_kernel_spmd(nc, in_maps, list(range(NCORES)), trace=trace)
    _cache["last_results"] = res

    out = np.empty((TOK, D), np.float32)
    for r in range(NCORES):
        o = np.asarray(res.results[r]["out"])
        for c, (roff, tokpc) in enumerate(CHUNKS):
            g0 = CHUNK_TOK0[c]
            out[g0 + r * tokpc: g0 + (r + 1) * tokpc] = \
                o[roff:roff + tokpc]
    out = out + np.asarray(bo, np.float32)[None, :]
    return out.reshape(B, S, D)


# revision 48
# speedup vs baseline: 1.2328x; 1.1055x over previous
"""Multi-head attention (B=2, S=2048, D=1024, H=16) on 8 NeuronCores.

Sharding: Megatron tensor parallelism. Core r owns heads 2r, 2r+1
(a 128-wide slice of D). Wq/Wk/Wv column-parallel. For the output
projection, attnT (each core's 128 D-rows, bf16) is exchanged in five
token-chunks with AllToAll so every core ends up with all 1024 D-rows
for a token slice; each core then multiplies by the full Wo and writes
its slice. Host reassembles the slices and adds bo.

All matmul operands are bf16 (1 cyc/row on the PE vs 4 for fp32);
PSUM accumulation stays fp32. Activations are converted to bf16 on
the host, halving HBM reads.

Scheduling notes:
 - attention is software-pipelined (scores for group n+1 are emitted
   before PV of group n) so exp never waits on the tensor queue;
 - batch 1's projections are interleaved piecewise into batch 0's
   attention stream, and the Wo chunks into batch 1's, keeping the PE
   continuously busy so it ramps to (and stays at) full clock;
 - weights load on the Activation DMA queue so the SP queue leads
   with the x tiles the first matmuls need.

Layouts on device (per core):
  xqT/xkT/xvT : [1024, 4096]  host-transposed bf16 activations
  qT/kT       : [128, 2048]   per batch, dk-major (rows = 2 heads)
  v           : [128, 130]    16 tok-tiles/batch; cols = [v_h0 | 1 | v_h1 | 1]
                              (ones column makes PV emit softmax sums)
  scores      : psum [128 sk, 1024] = 2 sk-tiles -> one Exp -> pt bf16
  PV          : psum [65, 512] accumulated over 16 sk tiles; row 64 = sums
  attnT       : [128, 2048]   normalized, per batch; AllToAll'd in chunks
"""

import sys

sys.path.insert(0, "/opt/trn_rl_repo")

import numpy as np

B, S, D, H, DK = 2, 2048, 1024, 16, 64
NCORES = 8
TOK = B * S            # 4096
DKC = D // NCORES      # 128 = 2 heads per core
TOKC = TOK // NCORES   # 512 output rows per core
KT = D // 128          # 8 contraction tiles
SKT = S // 128         # 16 key tiles per batch
SQB = S // 512         # 4 query blocks per batch
SKG = 2                # sk tiles per exp group
NG = SKT // SKG        # 8 exp groups per (head, query block)

# chunk c: (out_ext row offset, tokens per core). Chunks 0-2 cover 1024
# global tokens each, 3-4 cover 512 (the last exchange is small and the
# one before it overlaps the final attention block).
CHUNKS = [(0, 128), (128, 128), (256, 128), (384, 64), (448, 64)]
CHUNK_TOK0 = [0, 1024, 2048, 3072, 3584]

_cache = {}


def _build():
    from contextlib import ExitStack

    from concourse import bacc
    import concourse.mybir as mybir
    import concourse.tile as tile

    f32 = mybir.dt.float32
    bf16 = mybir.dt.bfloat16
    Act = mybir.ActivationFunctionType

    nc = bacc.Bacc(
        "TRN2", target_bir_lowering=False, debug=False,
        enable_asserts=False, num_devices=NCORES,
    )

    xqT = nc.dram_tensor("xqT", [D, TOK], bf16, kind="ExternalInput").ap()
    xkT = nc.dram_tensor("xkT", [D, TOK], bf16, kind="ExternalInput").ap()
    xvT = nc.dram_tensor("xvT", [D, TOK], bf16, kind="ExternalInput").ap()
    wq = nc.dram_tensor("wq", [D, DKC], bf16, kind="ExternalInput").ap()
    wk = nc.dram_tensor("wk", [D, DKC], bf16, kind="ExternalInput").ap()
    wv = nc.dram_tensor("wv", [D, DKC], bf16, kind="ExternalInput").ap()
    wo = nc.dram_tensor("wo", [D, D], bf16, kind="ExternalInput").ap()
    bq = nc.dram_tensor("bq", [DKC, 1], f32, kind="ExternalInput").ap()
    bk = nc.dram_tensor("bk", [DKC, 1], f32, kind="ExternalInput").ap()
    bv = nc.dram_tensor("bv", [1, DKC], bf16, kind="ExternalInput").ap()
    out_ext = nc.dram_tensor("out", [TOKC, D], f32, kind="ExternalOutput").ap()

    with tile.TileContext(nc) as tc, ExitStack() as ctx, \
            nc.allow_low_precision("bf16 matmul operands, fp32 psum accumulate"):
        wpool = ctx.enter_context(tc.tile_pool(name="w", bufs=1))
        xpool = ctx.enter_context(tc.tile_pool(name="x", bufs=2))
        qkpool = ctx.enter_context(tc.tile_pool(name="qk", bufs=2))
        vpool = ctx.enter_context(tc.tile_pool(name="v", bufs=2))
        ptpool = ctx.enter_context(tc.tile_pool(name="pt", bufs=3))
        atpool = ctx.enter_context(tc.tile_pool(name="at", bufs=2))
        smpool = ctx.enter_context(tc.tile_pool(name="sm", bufs=2))
        agpool = ctx.enter_context(tc.tile_pool(name="ag", bufs=1))
        opool = ctx.enter_context(tc.tile_pool(name="o", bufs=2))
        ps_g = ctx.enter_context(tc.tile_pool(name="psg", bufs=2, space="PSUM"))
        ps_mm = ctx.enter_context(tc.tile_pool(name="psmm", bufs=2, space="PSUM"))
        ps_acc = ctx.enter_context(tc.tile_pool(name="psacc", bufs=2, space="PSUM"))
        dram = ctx.enter_context(tc.tile_pool(name="dram", bufs=1, space="DRAM"))

        # ---- constants / weights into SBUF (Activation DMA queue) ----
        bq_t = wpool.tile([DKC, 1], f32, tag="bq")
        nc.scalar.dma_start(bq_t[:], bq[:])
        bk_t = wpool.tile([DKC, 1], f32, tag="bk")
        nc.scalar.dma_start(bk_t[:], bk[:])
        bv_t = wpool.tile([1, DKC], bf16, tag="bv")
        nc.scalar.dma_start(bv_t[:], bv[:])
        wq_t, wk_t, wv_t = [], [], []
        for name, src, lst in (("wq", wq, wq_t), ("wk", wk, wk_t), ("wv", wv, wv_t)):
            for k in range(KT):
                t = wpool.tile([128, DKC], bf16, tag=f"{name}{k}")
                nc.scalar.dma_start(t[:], src[k * 128:(k + 1) * 128, :])
                lst.append(t)
        wo_t = []
        for r in range(NCORES):
            t = wpool.tile([128, D], bf16, tag=f"wo{r}")
            nc.scalar.dma_start(t[:], wo[r * 128:(r + 1) * 128, :])
            wo_t.append(t)
        ones_t = wpool.tile([1, 128], bf16, tag="ones")
        nc.gpsimd.memset(ones_t[:], 1.0)
        ones_f = wpool.tile([1, 64], f32, tag="onesf")
        nc.gpsimd.memset(ones_f[:], 1.0)

        # ---------- emission helpers ----------

        def emit_x_load(xT, b, k, out_list):
            t = xpool.tile([128, S], bf16, tag=f"x{k}")
            nc.sync.dma_start(t[:], xT[k * 128:(k + 1) * 128, b * S:(b + 1) * S])
            out_list[k] = t

        def emit_qk_chain(chst, xts, w_list, bias_t, dst, blk, part):
            """Half of one 512-column projection chain (4 of 8 k-steps);
            part 0 allocates the psum tile, part 1 drains it."""
            if part == 0:
                chst[blk] = ps_mm.tile(
                    [128, 512], f32, tag="mm", name=f"chain{blk}")
            ps = chst[blk]
            for k in range(part * 4, part * 4 + 4):
                nc.tensor.matmul(
                    ps[:], lhsT=w_list[k][:],
                    rhs=xts[k][:, blk * 512:(blk + 1) * 512],
                    start=(k == 0), stop=(k == KT - 1),
                )
            if part == 1:
                nc.vector.tensor_scalar_add(
                    dst[:, blk * 512:(blk + 1) * 512], ps[:], bias_t[:, 0:1])

        def emit_v_tile(xvs, mi, v_tiles):
            ps = ps_mm.tile([128, DKC], f32, tag="mm")
            for k in range(KT):
                nc.tensor.matmul(
                    ps[:], lhsT=xvs[k][:, mi * 128:(mi + 1) * 128],
                    rhs=wv_t[k][:], start=(k == 0), stop=False,
                )
            nc.tensor.matmul(
                ps[:], lhsT=ones_t[0:1, :], rhs=bv_t[:],
                start=False, stop=True,
            )
            vt = vpool.tile([128, 130], bf16, tag=f"v{mi}")
            nc.vector.tensor_copy(vt[:, 0:64], ps[:, 0:64])
            nc.vector.tensor_copy(vt[:, 65:129], ps[:, 64:128])
            nc.vector.memset(vt[:, 64:65], 1.0)
            nc.vector.memset(vt[:, 129:130], 1.0)
            v_tiles[mi] = vt

        def proj_pieces(b, st):
            """Piecewise emission of batch b's x loads + q/k/v projections.
            Returns a list of (min_idx, thunk); thunks must run in order."""
            st["q"] = qkpool.tile([128, S], bf16, tag="qT", name=f"qT{b}")
            st["k"] = qkpool.tile([128, S], bf16, tag="kT", name=f"kT{b}")
            st["v"] = [None] * SKT
            xq_l, xk_l, xv_l = [None] * KT, [None] * KT, [None] * KT
            ps = []

            def loads(xT, lst):
                def mk(k):
                    return lambda: emit_x_load(xT, b, k, lst)
                return [mk(k) for k in range(KT)]

            chq, chk = {}, {}
            # k first: attention's first scores need ALL of kT but only
            # query block 0 of qT, so kT completion gates the lead-in
            ps += loads(xkT, xk_l)
            ps += loads(xqT, xq_l)
            for blk in range(SQB):
                for part in range(2):
                    ps.append(lambda blk=blk, part=part: emit_qk_chain(
                        chk, xk_l, wk_t, bk_t, st["k"], blk, part))
            ps += loads(xvT, xv_l)
            for blk in range(SQB):
                for part in range(2):
                    ps.append(lambda blk=blk, part=part: emit_qk_chain(
                        chq, xq_l, wq_t, bq_t, st["q"], blk, part))
            for mi in range(SKT):
                ps.append(lambda mi=mi: emit_v_tile(xv_l, mi, st["v"]))
            # spread pieces roughly one per attention group
            return [(j, f) for j, f in enumerate(ps)]

        def emit_exchange(c, attnT):
            tokpc = CHUNKS[c][1]
            loc0 = CHUNK_TOK0[c] % S
            ain = dram.tile([NCORES * 128, tokpc], bf16, tag=f"a2ai{c}")
            aout = dram.tile([NCORES * 128, tokpc], bf16, tag=f"a2ao{c}")
            for j in range(NCORES):
                nc.sync.dma_start(
                    ain[j * 128:(j + 1) * 128, :],
                    attnT[:, loc0 + j * tokpc: loc0 + (j + 1) * tokpc],
                )
            nc.gpsimd.collective_compute(
                "AllToAll",
                mybir.AluOpType.bypass,
                replica_groups=[list(range(NCORES))],
                ins=[ain.opt()],
                outs=[aout.opt()],
            )
            return aout

        def wo_pieces(c, aouts, st, nparts=2):
            """agt loads, then each output half as `nparts` psum chain
            parts (8/nparts accumulation matmuls each)."""
            roff, tokpc = CHUNKS[c]
            rstep = NCORES // nparts

            def p_load():
                agts = []
                for r in range(NCORES):
                    t = agpool.tile([128, tokpc], bf16, tag=f"ag{c}_{r}")
                    nc.sync.dma_start(
                        t[:], aouts[c][r * 128:(r + 1) * 128, :])
                    agts.append(t)
                st[c] = agts

            def p_half(half, part):
                agts = st[c]
                if part == 0:
                    st[(c, half)] = ps_mm.tile(
                        [tokpc, 512], f32, tag="mm", name=f"wo{c}_{half}")
                ps = st[(c, half)]
                for r in range(part * rstep, (part + 1) * rstep):
                    nc.tensor.matmul(
                        ps[:], lhsT=agts[r][:],
                        rhs=wo_t[r][:, half * 512:(half + 1) * 512],
                        start=(r == 0), stop=(r == NCORES - 1),
                    )
                if part == nparts - 1:
                    ot = opool.tile([tokpc, 512], f32, tag="ot")
                    nc.vector.tensor_copy(ot[:], ps[:])
                    nc.sync.dma_start(
                        out_ext[roff:roff + tokpc,
                                half * 512:(half + 1) * 512],
                        ot[:],
                    )

            return [p_load] + [
                (lambda half=half, part=part: p_half(half, part))
                for half in range(2) for part in range(nparts)]

        def emit_attention(b, qT_b, kT_b, v_tiles, filler):
            """Software-pipelined attention for batch b. `filler` is a list
            of (min_idx, thunk); thunks are popped in order, at most one
            per group, once idx >= min_idx."""
            attnT = atpool.tile([128, S], bf16, tag="attnT")
            items = [(sq, h, g)
                     for sq in range(SQB) for h in range(2) for g in range(NG)]

            def emit_scores(it):
                sq, h, g = it
                hp = h * 64
                qs = slice(sq * 512, (sq + 1) * 512)
                sg = ps_g.tile([128, 512 * SKG], f32, tag="sg")
                for i in range(SKG):
                    sk = g * SKG + i
                    nc.tensor.matmul(
                        sg[:, i * 512:(i + 1) * 512],
                        lhsT=kT_b[hp:hp + 64, sk * 128:(sk + 1) * 128],
                        rhs=qT_b[hp:hp + 64, qs],
                        start=True, stop=True,
                    )
                return sg

            sgs = {items[0]: emit_scores(items[0])}
            xps = None
            for idx, it in enumerate(items):
                sq, h, g = it
                hp = h * 64
                if idx + 1 < len(items):
                    sgs[items[idx + 1]] = emit_scores(items[idx + 1])
                sg = sgs.pop(it)
                ptg = ptpool.tile([128, 512 * SKG], bf16, tag="pt")
                nc.scalar.activation(ptg[:], sg[:], Act.Exp, scale=0.125)
                # filler slots in here: it runs on the tensor engine while
                # ACT computes the exp that the PV pair below waits on
                while filler and idx >= filler[0][0]:
                    filler.pop(0)[1]()
                if g == 0:
                    xps = ps_acc.tile([65, 512], f32, tag="acc")
                for i in range(SKG):
                    sk = g * SKG + i
                    nc.tensor.matmul(
                        xps[:],
                        lhsT=v_tiles[sk][:, h * 65:h * 65 + 65],
                        rhs=ptg[:, i * 512:(i + 1) * 512],
                        start=(sk == 0), stop=(sk == SKT - 1),
                    )
                if g == NG - 1:
                    # bf16 broadcast matmul (1 cyc/row vs 4 for fp32); the
                    # bf16-rounded denominator adds <1e-3 to the error
                    rowsum = smpool.tile([1, 512], bf16, tag="rs")
                    nc.scalar.activation(rowsum[:], xps[64:65, :], Act.Identity)
                    rbp = ps_mm.tile([64, 512], f32, tag="mm")
                    nc.tensor.matmul(
                        rbp[:], lhsT=ones_t[0:1, 0:64], rhs=rowsum[:],
                        start=True, stop=True,
                    )
                    rb = smpool.tile([64, 512], f32, tag="rb")
                    nc.vector.reciprocal_approx_fast(rb[:], rbp[:])
                    nc.vector.tensor_mul(
                        attnT[hp:hp + 64, sq * 512:(sq + 1) * 512],
                        xps[0:64, :], rb[:],
                    )
                    if h == 1 and (b, sq) in EXCH_AFTER:
                        for c in EXCH_AFTER[(b, sq)]:
                            aouts[c] = emit_exchange(c, attnT)
            # drain leftover filler
            for _, f in filler:
                f()
            filler.clear()

        # exchange c is emitted right after the attention block that
        # completes its tokens: c0 after (b0,sq1), c1 after (b0,sq3),
        # c2 after (b1,sq1), c3 after (b1,sq2), c4 after (b1,sq3).
        EXCH_AFTER = {(0, 1): [0], (0, 3): [1],
                      (1, 1): [2], (1, 2): [3], (1, 3): [4]}
        aouts = {}

        # ---- batch 0 x loads + q/k projections (inline lead-in); its
        # v projection runs as early just-in-time filler inside its own
        # attention (PV of group g needs v tile 2g+1, and two v pieces
        # are popped per group), followed by batch 1's pieces ----
        st0 = {}
        p0 = proj_pieces(0, st0)
        nv = SKT  # trailing SKT pieces of proj_pieces are the v tiles
        for _, f in p0[:-nv]:
            f()
        st1 = {}
        filler0 = [(j // 2, f) for j, (_, f) in enumerate(p0[-nv:])]
        filler0 += [(8 + j, f) for j, (_, f) in enumerate(proj_pieces(1, st1))]
        emit_attention(0, st0["q"], st0["k"], st0["v"], filler0)

        # ---- batch 1 attention, interleaving the Wo chunks as fine
        # 2-matmul pieces: spreading them over more groups keeps the PE
        # continuously busy so it holds full clock through the window ----
        agst = {}
        filler = []
        for j, f in enumerate(wo_pieces(0, aouts, agst, nparts=4)):
            filler.append((j * 2, f))
        for j, f in enumerate(wo_pieces(1, aouts, agst, nparts=4)):
            filler.append((18 + j * 2, f))
        # chunk 2's exchange is emitted after (b1, sq1) = idx 31
        for j, f in enumerate(wo_pieces(2, aouts, agst, nparts=4)):
            filler.append((40 + j * 2, f))
        emit_attention(1, st1["q"], st1["k"], st1["v"], filler)

        # ---- last two chunks' output projections (tail; chunk 3's
        # collective completed during the final attention block) ----
        for f in wo_pieces(3, aouts, agst):
            f()
        for f in wo_pieces(4, aouts, agst):
            f()

    nc.compile()
    return nc


def _get_nc():
    if "nc" not in _cache:
        _cache["nc"] = _build()
    return _cache["nc"]


def kernel(query, key, value, Wq, bq, Wk, bk, Wv, bv, Wo, bo, trace=False):
    import ml_dtypes
    from concourse.bass_utils import run_bass_kernel_spmd

    bf = ml_dtypes.bfloat16
    nc = _get_nc()

    q = np.ascontiguousarray(
        np.asarray(query, np.float32).reshape(TOK, D).T.astype(bf))
    k = np.ascontiguousarray(
        np.asarray(key, np.float32).reshape(TOK, D).T.astype(bf))
    v = np.ascontiguousarray(
        np.asarray(value, np.float32).reshape(TOK, D).T.astype(bf))
    Wq = np.asarray(Wq, np.float32)
    Wk = np.asarray(Wk, np.float32)
    Wv = np.asarray(Wv, np.float32)
    Wo_b = np.ascontiguousarray(np.asarray(Wo, np.float32).astype(bf))

    in_maps = []
    for r in range(NCORES):
        sl = slice(r * DKC, (r + 1) * DKC)
        in_maps.append({
            "xqT": q, "xkT": k, "xvT": v,
            "wq": np.ascontiguousarray(Wq[:, sl].astype(bf)),
            "wk": np.ascontiguousarray(Wk[:, sl].astype(bf)),
            "wv": np.ascontiguousarray(Wv[:, sl].astype(bf)),
            "wo": Wo_b,
            "bq": np.ascontiguousarray(np.asarray(bq, np.float32)[sl, None]),
            "bk": np.ascontiguousarray(np.asarray(bk, np.float32)[sl, None]),
            "bv": np.ascontiguousarray(
                np.asarray(bv, np.float32)[None, sl].astype(bf)),
        })

    res = run_bass_kernel_spmd(nc, in_maps, list(range(NCORES)), trace=trace)
    _cache["last_results"] = res

    out = np.empty((TOK, D), np.float32)
    for r in range(NCORES):
        o = np.asarray(res.results[r]["out"])
        for c, (roff, tokpc) in enumerate(CHUNKS):
            g0 = CHUNK_TOK0[c]
            out[g0 + r * tokpc: g0 + (r + 1) * tokpc] = \
                o[roff:roff + tokpc]
    out = out + np.asarray(bo, np.float32)[None, :]
    return out.reshape(B, S, D)
